# revision 13
# baseline (speedup 1.0000x reference)
"""Trainium2 Bass kernel for nn_Attention_Layer_64364379898508.

Pipeline (per core, data-parallel over B=4096 -> 8 x 512):
  reference:  info = [q, k, q-k, q*k] @ W1 -> relu -> @W2 -> relu -> @Wf
              -> masked softmax over T -> attn-weighted sum of v.
  algebra:    info@W1 = q@(W1a+W1c) + k@(W1b-W1c) + (q*k)@W1d
              => z1 = Wstack.T @ [k_fm; qk_fm]  (K=128 matmul)
                    + [Wq; b1].T @ [q_rep; 1]   (K=65 matmul, broadcast rhs)
  host prep:  k transposed to feature-major, qk = q*k precomputed, both bf16,
              T padded 200->256 (pad masked out), v bf16, all per-core shards.
  logits:     Wf folded into a [104,2] dual-column matmul over partition-packed
              h2 pairs; escaped via one wide ACT copy; reshaped to [b, t] by
              SBUF->SBUF DMA; softmax rows; attn transposed back via PE.
  v-sum:      per-(b, t-chunk) matmul: lhsT = v chunk [128,64], rhs = attn
              column [128,1], accumulated into a [64, 128] psum per group.
"""
import numpy as np
import ml_dtypes

import concourse.bacc as bacc
import concourse.mybir as mybir
from concourse.tile import TileContext, add_dep_helper
from concourse.bass_utils import run_bass_kernel_spmd

F32 = mybir.dt.float32
BF16 = mybir.dt.bfloat16
AF = mybir.ActivationFunctionType
ALU = mybir.AluOpType

B, T, D = 4096, 200, 64
H1, H2 = 80, 40
NCORES = 8
BC = B // NCORES          # 512 b per core
TP = 256                  # padded T
NG = 4                    # groups of 128 b per core
GB = 128                  # b per group

_cache = {}
DEBUG = False


def _build_program():
    nc = bacc.Bacc()

    kq_in = nc.dram_tensor("kq", [BC, 128, TP], BF16, kind="ExternalInput")
    v_in = nc.dram_tensor("v", [BC, TP, D], BF16, kind="ExternalInput")
    mf_in = nc.dram_tensor("mf", [BC, TP], F32, kind="ExternalInput")
    q1_in = nc.dram_tensor("q1", [65, BC], BF16, kind="ExternalInput")
    ws_in = nc.dram_tensor("ws", [128, H1], BF16, kind="ExternalInput")
    wq_in = nc.dram_tensor("wq", [65, H1], BF16, kind="ExternalInput")
    w2_in = nc.dram_tensor("w2", [H1, 64], BF16, kind="ExternalInput")
    wf_in = nc.dram_tensor("wf", [104, 32], BF16, kind="ExternalInput")
    b2_in = nc.dram_tensor("b2", [128, 1], F32, kind="ExternalInput")
    id_in = nc.dram_tensor("idm", [128, 128], BF16, kind="ExternalInput")
    out_t = nc.dram_tensor("ofm", [D, BC], F32, kind="ExternalOutput")
    dbg_lbt = nc.dram_tensor("dbg_lbt", [GB, TP], F32, kind="ExternalOutput") if DEBUG else None
    dbg_lsb = nc.dram_tensor("dbg_lsb", [128, 512], F32, kind="ExternalOutput") if DEBUG else None
    dbg_at = nc.dram_tensor("dbg_at", [GB, TP], F32, kind="ExternalOutput") if DEBUG else None
    dbg_afm = nc.dram_tensor("dbg_afm", [128, 256], F32, kind="ExternalOutput") if DEBUG else None

    with TileContext(nc) as tc:
        with tc.tile_pool(name="const", bufs=1) as cp, \
             tc.tile_pool(name="io", bufs=3) as iop, \
             tc.tile_pool(name="act", bufs=3) as ap, \
             tc.tile_pool(name="sm", bufs=2) as sp, \
             tc.tile_pool(name="z1p", bufs=2, space="PSUM") as z1p, \
             tc.tile_pool(name="z2p", bufs=2, space="PSUM") as z2p, \
             tc.tile_pool(name="lfp", bufs=1, space="PSUM") as lfp, \
             tc.tile_pool(name="vp", bufs=1, space="PSUM") as vpp:
            ws = cp.tile([128, H1], BF16)
            wq = cp.tile([65, H1], BF16)
            w2 = cp.tile([H1, 64], BF16)
            wf = cp.tile([104, 32], BF16)
            b2d = cp.tile([128, 1], F32)
            q1 = cp.tile([65, BC], BF16)
            idm = cp.tile([128, 128], BF16)
            ofm = cp.tile([D, BC], F32)
            nc.gpsimd.dma_start(out=ws[:], in_=ws_in[:, :])
            nc.gpsimd.dma_start(out=wq[:], in_=wq_in[:, :])
            nc.gpsimd.dma_start(out=w2[:], in_=w2_in[:, :])
            nc.gpsimd.dma_start(out=wf[:], in_=wf_in[:, :])
            nc.gpsimd.dma_start(out=b2d[:], in_=b2_in[:, :])
            nc.gpsimd.dma_start(out=q1[:], in_=q1_in[:, :])
            nc.gpsimd.dma_start(out=idm[:], in_=id_in[:, :])

            for g in range(NG):
                b0 = g * GB
                # mask for this group, rows pre-permuted host-side to lbt order
                mfg = sp.tile([GB, TP], F32, name="mfg")
                nc.gpsimd.dma_start(
                    out=mfg[:], in_=mf_in[b0:b0 + GB, :])

                # ---- MLP + logits over 8 lf-units of 16 b ----
                lbt = sp.tile([GB, TP], F32, name="lbt")
                prev_rd = None
                for u in range(8):
                    ub = b0 + u * 16
                    # load kq for 16 b: [128, 16*256]
                    kqt = iop.tile([128, 16 * TP], BF16, name="kqt", tag="kqt")
                    nc.sync.dma_start(
                        out=kqt[:].rearrange("p (b t) -> p b t", b=16),
                        in_=kq_in[ub:ub + 16].rearrange("b p t -> p b t"))

                    lps = lfp.tile([128, 512], F32, name="lps", tag="lps")
                    h2s = []
                    for pr in range(2):   # pairs of quads (8 b)
                        h1s = []
                        for qd in range(2):   # quads (4 b)
                            qb = pr * 8 + qd * 4   # b offset in unit
                            z1 = z1p.tile([H1, 1024], F32, name="z1", tag="z1")
                            for w in range(2):    # 512-col windows
                                cols = slice(qb * TP + w * 512, qb * TP + w * 512 + 512)
                                zw = z1[:, w * 512:(w + 1) * 512]
                                nc.tensor.matmul(zw, ws[:], kqt[:, cols],
                                                 start=True, stop=False)
                                bb = ub + qb + w * 2
                                rhs = q1[:, bb:bb + 2].unsqueeze(2) \
                                    .broadcast_to([65, 2, TP])
                                nc.tensor.matmul(
                                    zw.rearrange("p (b t) -> p b t", b=2),
                                    wq[:], rhs, start=False, stop=True)
                            h1 = ap.tile([H1, 1024], BF16, name="h1", tag="h1")
                            nc.scalar.activation(out=h1[:, 0:512], in_=z1[:, 0:512],
                                                 func=AF.Relu)
                            nc.vector.tensor_scalar_max(
                                out=h1[:, 512:1024], in0=z1[:, 512:1024], scalar1=0.0)
                            h1s.append(h1)
                        # L2: pack pair into [104, 512] x 2 windows
                        for w in range(2):
                            z2 = z2p.tile([104, 512], F32, name="z2", tag="z2")
                            nc.tensor.matmul(z2[0:64, :], w2[:],
                                             h1s[0][:, w * 512:(w + 1) * 512],
                                             start=True, stop=True,
                                             tile_position=(0, 0))
                            nc.tensor.matmul(z2[64:104, :], w2[:, 0:40],
                                             h1s[1][:, w * 512:(w + 1) * 512],
                                             start=True, stop=True,
                                             tile_position=(0, 64))
                            h2 = ap.tile([104, 512], BF16, name="h2", tag="h2")
                            nc.scalar.activation(out=h2[0:64, :], in_=z2[0:64, :],
                                                 func=AF.Relu, bias=b2d[0:64, :])
                            nc.vector.tensor_scalar(
                                out=h2[64:104, :], in0=z2[64:104, :],
                                scalar1=b2d[64:104, :], scalar2=0.0,
                                op0=ALU.add, op1=ALU.max)
                            # Lf: K=104 -> [32, 512] at offset 32*(2*pr+w)
                            # (wf cols 2:32 zero -> psum rows zero-filled;
                            #  h2 rows 40:64 are exact zeros via padded W2)
                            o = 32 * (2 * pr + w)
                            nc.tensor.matmul(lps[o:o + 32, :], wf[:], h2[:],
                                             start=True, stop=True,
                                             tile_position=(0, o))
                    lsb = sp.tile([128, 512], F32, name="lsb", tag="lsb")
                    nc.scalar.copy(out=lsb[:], in_=lps[:])
                    if DEBUG and g == 0 and u == 0:
                        nc.sync.dma_start(out=dbg_lsb[:, :], in_=lsb[:])
                    for m in range(4):
                        # src rows (32m, 32m+1) adjacent; dst d-local = 4m+2h+cb
                        srcap = lsb[32 * m:32 * m + 2, :] \
                            .rearrange("h (cb t) -> h cb t", cb=2)
                        dst = lbt[u * 16 + 4 * m: u * 16 + 4 * m + 4, :]
                        rd = nc.gpsimd.dma_start(out=dst, in_=srcap)
                        if prev_rd is not None:
                            add_dep_helper(rd.ins, prev_rd.ins,
                                           reason="serialize lbt reshape writes")
                        prev_rd = rd

                if DEBUG and g == 0:
                    nc.sync.dma_start(out=dbg_lbt[:, :], in_=lbt[:])
                # ---- softmax over t ----
                ex = sp.tile([GB, TP], F32, name="ex")
                nc.scalar.activation(out=ex[:], in_=lbt[:], func=AF.Exp)
                em = sp.tile([GB, TP], F32, name="em")
                nc.vector.tensor_mul(out=em[:], in0=ex[:], in1=mfg[:])
                sm = sp.tile([GB, 1], F32, name="sm")
                nc.vector.tensor_reduce(out=sm[:], in_=em[:],
                                        axis=mybir.AxisListType.X, op=ALU.add)
                rc = sp.tile([GB, 1], F32, name="rc")
                nc.vector.reciprocal(out=rc[:], in_=sm[:])
                at = sp.tile([GB, TP], BF16, name="at")
                nc.vector.tensor_scalar_mul(out=at[:], in0=em[:], scalar1=rc[:])

                if DEBUG and g == 0:
                    dat = sp.tile([GB, TP], F32, name="dat")
                    nc.vector.tensor_copy(out=dat[:], in_=at[:])
                    nc.sync.dma_start(out=dbg_at[:, :], in_=dat[:])
                # ---- transpose attn to [t, b] via PE ----
                afm = sp.tile([128, 256], BF16, name="afm")
                tp1 = z1p.tile([128, 256], BF16, name="tp1", tag="z1")
                for c in range(2):
                    nc.tensor.transpose(tp1[:, c * 128:(c + 1) * 128],
                                        at[:, c * 128:(c + 1) * 128], idm[:])
                nc.scalar.copy(out=afm[:], in_=tp1[:])

                if DEBUG and g == 0:
                    dafm = sp.tile([128, 256], F32, name="dafm")
                    nc.vector.tensor_copy(out=dafm[:], in_=afm[:])
                    nc.sync.dma_start(out=dbg_afm[:, :], in_=dafm[:])
                # ---- v-sum: per (b, chunk) matmul accumulating [64, 128] ----
                vps = vpp.tile([D, GB], F32, name="vps", tag="vps")
                for w in range(8):   # v tiles of 16 b
                    vt = iop.tile([128, 16 * 128], BF16, name="vt", tag="vt")
                    nc.scalar.dma_start(
                        out=vt[:].rearrange("p (b c d) -> p b c d", b=16, c=2),
                        in_=v_in[b0 + w * 16: b0 + w * 16 + 16]
                        .rearrange("b (c p) d -> p b c d", c=2))
                    for j in range(16):
                        bl = w * 16 + j          # b index in group (true order)
                        # lbt/afm row for this b (permuted): r = 8pr+4h+2wn+cb
                        # maps to d = 8h + 4pr + 2wn + cb within the 16-b unit
                        u, r = bl // 16, bl % 16
                        pr, h, wn, cb = r // 8, (r // 4) % 2, (r // 2) % 2, r % 2
                        d_row = u * 16 + 8 * pr + 4 * wn + 2 * h + cb
                        for c in range(2):
                            nc.tensor.matmul(
                                vps[:, bl:bl + 1],
                                vt[:, j * 128 + c * 64: j * 128 + c * 64 + 64],
                                afm[:, c * 128 + d_row: c * 128 + d_row + 1],
                                start=(c == 0), stop=(c == 1))
                nc.scalar.copy(out=ofm[:, b0:b0 + GB], in_=vps[:])

            nc.sync.dma_start(out=out_t[:, :], in_=ofm[:])
    nc.compile()
    return nc


def _lbt_perm():
    """Permutation: lbt row d  ->  group-local b index."""
    perm = np.zeros(GB, dtype=np.int64)
    for d in range(GB):
        u, dl = d // 16, d % 16
        pr, wn, h, cb = (dl // 8) % 2, (dl // 4) % 2, (dl // 2) % 2, dl % 2
        r = 8 * pr + 4 * h + 2 * wn + cb
        perm[d] = u * 16 + r
    return perm


def _host_prep(q, k, v, mask, W1, b1, W2, b2, Wf, bf):
    bf16 = ml_dtypes.bfloat16
    W1a, W1b = W1[0:D], W1[D:2 * D]
    W1c, W1d = W1[2 * D:3 * D], W1[3 * D:4 * D]
    ws = np.concatenate([W1b - W1c, W1d], axis=0).astype(bf16)       # [128, 80]
    wq = np.concatenate([W1a + W1c, b1[None, :]], axis=0).astype(bf16)  # [65, 80]
    w2 = np.zeros((H1, 64), dtype=np.float32)
    w2[:, 0:40] = W2
    w2 = w2.astype(bf16)
    wfd = np.zeros((104, 32), dtype=np.float32)
    wfd[0:40, 0] = Wf[:, 0]
    wfd[64:104, 1] = Wf[:, 0]
    wfd = wfd.astype(bf16)
    b2d = np.zeros((128, 1), dtype=np.float32)
    b2d[0:40, 0] = b2
    b2d[64:104, 0] = b2
    idm = np.eye(128, dtype=np.float32).astype(bf16)

    k_fm = np.zeros((B, D, TP), dtype=np.float32)
    k_fm[:, :, :T] = k.transpose(0, 2, 1)
    qk_fm = k_fm * q[:, :, None]
    kq = np.concatenate([k_fm, qk_fm], axis=1).astype(bf16)          # [B, 128, 256]
    vp = np.zeros((B, TP, D), dtype=np.float32)
    vp[:, :T, :] = v
    vp = vp.astype(bf16)
    mfp = np.zeros((B, TP), dtype=np.float32)
    mfp[:, :T] = (mask != 0).astype(np.float32)

    perm = _lbt_perm()
    in_maps = []
    for c in range(NCORES):
        s = slice(c * BC, (c + 1) * BC)
        q1 = np.concatenate(
            [q[s].T, np.ones((1, BC), np.float32)], axis=0).astype(bf16)
        mfc = mfp[s].reshape(NG, GB, TP)[:, perm, :].reshape(BC, TP)
        in_maps.append({
            "kq": np.ascontiguousarray(kq[s]),
            "v": np.ascontiguousarray(vp[s]),
            "mf": np.ascontiguousarray(mfc),
            "q1": np.ascontiguousarray(q1),
            "ws": ws, "wq": wq, "w2": w2, "wf": wfd, "b2": b2d, "idm": idm,
        })
    return in_maps


def kernel(q, k, v, mask, W1, b1, W2, b2, Wf, bf, _trace=False):
    q = np.asarray(q, np.float32)
    k = np.asarray(k, np.float32)
    v = np.asarray(v, np.float32)
    mask = np.asarray(mask)
    in_maps = _host_prep(q, k, v, mask,
                         np.asarray(W1, np.float32), np.asarray(b1, np.float32),
                         np.asarray(W2, np.float32), np.asarray(b2, np.float32),
                         np.asarray(Wf, np.float32), np.asarray(bf, np.float32))
    if "nc" not in _cache:
        _cache["nc"] = _build_program()
    r = run_bass_kernel_spmd(_cache["nc"], in_maps,
                             core_ids=list(range(NCORES)), trace=_trace)
    out = np.concatenate([r.results[c]["ofm"].T for c in range(NCORES)], axis=0)
    if _trace:
        kernel.last_exec_ns = r.exec_time_ns
        kernel.last_results = r
    return out.astype(np.float32)


# revision 14
# speedup vs baseline: 1.0643x; 1.0643x over previous
"""Trainium2 Bass kernel for nn_Attention_Layer_64364379898508.

Pipeline (per core, data-parallel over B=4096 -> 8 x 512):
  reference:  info = [q, k, q-k, q*k] @ W1 -> relu -> @W2 -> relu -> @Wf
              -> masked softmax over T -> attn-weighted sum of v.
  algebra:    info@W1 = q@(W1a+W1c) + k@(W1b-W1c) + (q*k)@W1d
              => z1 = Wstack.T @ [k_fm; qk_fm]  (K=128 matmul)
                    + [Wq; b1].T @ [q_rep; 1]   (K=65 matmul, broadcast rhs)
  host prep:  k transposed to feature-major, qk = q*k precomputed, both bf16,
              T padded 200->256 (pad masked out), v bf16, all per-core shards.
  logits:     Wf folded into a [104,2] dual-column matmul over partition-packed
              h2 pairs; escaped via one wide ACT copy; reshaped to [b, t] by
              SBUF->SBUF DMA; softmax rows; attn transposed back via PE.
  v-sum:      per-(b, t-chunk) matmul: lhsT = v chunk [128,64], rhs = attn
              column [128,1], accumulated into a [64, 128] psum per group.
"""
import numpy as np
import ml_dtypes

import concourse.bacc as bacc
import concourse.mybir as mybir
from concourse.tile import TileContext, add_dep_helper
from concourse.bass_utils import run_bass_kernel_spmd

F32 = mybir.dt.float32
BF16 = mybir.dt.bfloat16
AF = mybir.ActivationFunctionType
ALU = mybir.AluOpType

B, T, D = 4096, 200, 64
H1, H2 = 80, 40
NCORES = 8
BC = B // NCORES          # 512 b per core
TP = 256                  # padded T
NG = 4                    # groups of 128 b per core
GB = 128                  # b per group

_cache = {}
DEBUG = False


def _build_program():
    nc = bacc.Bacc()

    kq_in = nc.dram_tensor("kq", [32, 128, 16 * TP], BF16, kind="ExternalInput")
    v_in = nc.dram_tensor("v", [32, 128, 16 * 128], BF16, kind="ExternalInput")
    mf_in = nc.dram_tensor("mf", [BC, TP], F32, kind="ExternalInput")
    q1_in = nc.dram_tensor("q1", [65, BC], BF16, kind="ExternalInput")
    ws_in = nc.dram_tensor("ws", [128, H1], BF16, kind="ExternalInput")
    wq_in = nc.dram_tensor("wq", [65, H1], BF16, kind="ExternalInput")
    w2_in = nc.dram_tensor("w2", [H1, 64], BF16, kind="ExternalInput")
    wf_in = nc.dram_tensor("wf", [104, 32], BF16, kind="ExternalInput")
    b2_in = nc.dram_tensor("b2", [128, 1], F32, kind="ExternalInput")
    id_in = nc.dram_tensor("idm", [128, 128], BF16, kind="ExternalInput")
    out_t = nc.dram_tensor("ofm", [D, BC], F32, kind="ExternalOutput")
    dbg_lbt = nc.dram_tensor("dbg_lbt", [GB, TP], F32, kind="ExternalOutput") if DEBUG else None
    dbg_lsb = nc.dram_tensor("dbg_lsb", [128, 512], F32, kind="ExternalOutput") if DEBUG else None
    dbg_at = nc.dram_tensor("dbg_at", [GB, TP], F32, kind="ExternalOutput") if DEBUG else None
    dbg_afm = nc.dram_tensor("dbg_afm", [128, 256], F32, kind="ExternalOutput") if DEBUG else None

    with TileContext(nc) as tc:
        with tc.tile_pool(name="const", bufs=1) as cp, \
             tc.tile_pool(name="io", bufs=3) as iop, \
             tc.tile_pool(name="act", bufs=3) as ap, \
             tc.tile_pool(name="sm", bufs=2) as sp, \
             tc.tile_pool(name="z1p", bufs=2, space="PSUM") as z1p, \
             tc.tile_pool(name="z2p", bufs=2, space="PSUM") as z2p, \
             tc.tile_pool(name="lfp", bufs=1, space="PSUM") as lfp, \
             tc.tile_pool(name="vp", bufs=1, space="PSUM") as vpp:
            ws = cp.tile([128, H1], BF16)
            wq = cp.tile([65, H1], BF16)
            w2 = cp.tile([H1, 64], BF16)
            wf = cp.tile([104, 32], BF16)
            b2d = cp.tile([128, 1], F32)
            q1 = cp.tile([65, BC], BF16)
            idm = cp.tile([128, 128], BF16)
            ofm = cp.tile([D, BC], F32)
            nc.gpsimd.dma_start(out=ws[:], in_=ws_in[:, :])
            nc.gpsimd.dma_start(out=wq[:], in_=wq_in[:, :])
            nc.gpsimd.dma_start(out=w2[:], in_=w2_in[:, :])
            nc.gpsimd.dma_start(out=wf[:], in_=wf_in[:, :])
            nc.gpsimd.dma_start(out=b2d[:], in_=b2_in[:, :])
            nc.gpsimd.dma_start(out=q1[:], in_=q1_in[:, :])
            nc.gpsimd.dma_start(out=idm[:], in_=id_in[:, :])

            for g in range(NG):
                b0 = g * GB
                # mask for this group, rows pre-permuted host-side to lbt order
                mfg = sp.tile([GB, TP], F32, name="mfg")
                nc.gpsimd.dma_start(
                    out=mfg[:], in_=mf_in[b0:b0 + GB, :])

                # ---- MLP + logits over 8 lf-units of 16 b ----
                lbt = sp.tile([GB, TP], F32, name="lbt")
                prev_rd = None
                for u in range(8):
                    ub = b0 + u * 16
                    # load kq for 16 b: [128, 16*256]
                    kqt = iop.tile([128, 16 * TP], BF16, name="kqt", tag="kqt")
                    nc.sync.dma_start(out=kqt[:], in_=kq_in[ub // 16])

                    lps = lfp.tile([128, 512], F32, name="lps", tag="lps")
                    h2s = []
                    for pr in range(2):   # pairs of quads (8 b)
                        h1s = []
                        for qd in range(2):   # quads (4 b)
                            qb = pr * 8 + qd * 4   # b offset in unit
                            z1 = z1p.tile([H1, 1024], F32, name="z1", tag="z1")
                            for w in range(2):    # 512-col windows
                                cols = slice(qb * TP + w * 512, qb * TP + w * 512 + 512)
                                zw = z1[:, w * 512:(w + 1) * 512]
                                nc.tensor.matmul(zw, ws[:], kqt[:, cols],
                                                 start=True, stop=False)
                                bb = ub + qb + w * 2
                                rhs = q1[:, bb:bb + 2].unsqueeze(2) \
                                    .broadcast_to([65, 2, TP])
                                nc.tensor.matmul(
                                    zw.rearrange("p (b t) -> p b t", b=2),
                                    wq[:], rhs, start=False, stop=True)
                            h1 = ap.tile([H1, 1024], BF16, name="h1", tag="h1")
                            nc.scalar.activation(out=h1[:, 0:512], in_=z1[:, 0:512],
                                                 func=AF.Relu)
                            nc.vector.tensor_scalar_max(
                                out=h1[:, 512:1024], in0=z1[:, 512:1024], scalar1=0.0)
                            h1s.append(h1)
                        # L2: pack pair into [104, 512] x 2 windows
                        for w in range(2):
                            z2 = z2p.tile([104, 512], F32, name="z2", tag="z2")
                            nc.tensor.matmul(z2[0:64, :], w2[:],
                                             h1s[0][:, w * 512:(w + 1) * 512],
                                             start=True, stop=True,
                                             tile_position=(0, 0))
                            nc.tensor.matmul(z2[64:104, :], w2[:, 0:40],
                                             h1s[1][:, w * 512:(w + 1) * 512],
                                             start=True, stop=True,
                                             tile_position=(0, 64))
                            h2 = ap.tile([104, 512], BF16, name="h2", tag="h2")
                            nc.scalar.activation(out=h2[0:64, :], in_=z2[0:64, :],
                                                 func=AF.Relu, bias=b2d[0:64, :])
                            nc.vector.tensor_scalar(
                                out=h2[64:104, :], in0=z2[64:104, :],
                                scalar1=b2d[64:104, :], scalar2=0.0,
                                op0=ALU.add, op1=ALU.max)
                            # Lf: K=104 -> [32, 512] at offset 32*(2*pr+w)
                            # (wf cols 2:32 zero -> psum rows zero-filled;
                            #  h2 rows 40:64 are exact zeros via padded W2)
                            o = 32 * (2 * pr + w)
                            nc.tensor.matmul(lps[o:o + 32, :], wf[:], h2[:],
                                             start=True, stop=True,
                                             tile_position=(0, o))
                    lsb = sp.tile([128, 512], F32, name="lsb", tag="lsb")
                    nc.scalar.copy(out=lsb[:], in_=lps[:])
                    if DEBUG and g == 0 and u == 0:
                        nc.sync.dma_start(out=dbg_lsb[:, :], in_=lsb[:])
                    for m in range(4):
                        # src rows (32m, 32m+1) adjacent; dst d-local = 4m+2h+cb
                        srcap = lsb[32 * m:32 * m + 2, :] \
                            .rearrange("h (cb t) -> h cb t", cb=2)
                        dst = lbt[u * 16 + 4 * m: u * 16 + 4 * m + 4, :]
                        rd = nc.gpsimd.dma_start(out=dst, in_=srcap)
                        if prev_rd is not None:
                            add_dep_helper(rd.ins, prev_rd.ins,
                                           reason="serialize lbt reshape writes")
                        prev_rd = rd

                if DEBUG and g == 0:
                    nc.sync.dma_start(out=dbg_lbt[:, :], in_=lbt[:])
                # ---- softmax over t ----
                ex = sp.tile([GB, TP], F32, name="ex")
                nc.scalar.activation(out=ex[:], in_=lbt[:], func=AF.Exp)
                em = sp.tile([GB, TP], F32, name="em")
                nc.vector.tensor_mul(out=em[:], in0=ex[:], in1=mfg[:])
                sm = sp.tile([GB, 1], F32, name="sm")
                nc.vector.tensor_reduce(out=sm[:], in_=em[:],
                                        axis=mybir.AxisListType.X, op=ALU.add)
                rc = sp.tile([GB, 1], F32, name="rc")
                nc.vector.reciprocal(out=rc[:], in_=sm[:])
                at = sp.tile([GB, TP], BF16, name="at")
                nc.vector.tensor_scalar_mul(out=at[:], in0=em[:], scalar1=rc[:])

                if DEBUG and g == 0:
                    dat = sp.tile([GB, TP], F32, name="dat")
                    nc.vector.tensor_copy(out=dat[:], in_=at[:])
                    nc.sync.dma_start(out=dbg_at[:, :], in_=dat[:])
                # ---- transpose attn to [t, b] via PE ----
                afm = sp.tile([128, 256], BF16, name="afm")
                tp1 = z1p.tile([128, 256], BF16, name="tp1", tag="z1")
                for c in range(2):
                    nc.tensor.transpose(tp1[:, c * 128:(c + 1) * 128],
                                        at[:, c * 128:(c + 1) * 128], idm[:])
                nc.scalar.copy(out=afm[:], in_=tp1[:])

                if DEBUG and g == 0:
                    dafm = sp.tile([128, 256], F32, name="dafm")
                    nc.vector.tensor_copy(out=dafm[:], in_=afm[:])
                    nc.sync.dma_start(out=dbg_afm[:, :], in_=dafm[:])
                # ---- v-sum: per (b, chunk) matmul accumulating [64, 128] ----
                vps = vpp.tile([D, GB], F32, name="vps", tag="vps")
                for w in range(8):   # v tiles of 16 b
                    vt = iop.tile([128, 16 * 128], BF16, name="vt", tag="vt")
                    nc.scalar.dma_start(out=vt[:], in_=v_in[(b0 + w * 16) // 16])
                    for j in range(16):
                        bl = w * 16 + j          # b index in group (true order)
                        # lbt/afm row for this b (permuted): r = 8pr+4h+2wn+cb
                        # maps to d = 8h + 4pr + 2wn + cb within the 16-b unit
                        u, r = bl // 16, bl % 16
                        pr, h, wn, cb = r // 8, (r // 4) % 2, (r // 2) % 2, r % 2
                        d_row = u * 16 + 8 * pr + 4 * wn + 2 * h + cb
                        for c in range(2):
                            nc.tensor.matmul(
                                vps[:, bl:bl + 1],
                                vt[:, j * 128 + c * 64: j * 128 + c * 64 + 64],
                                afm[:, c * 128 + d_row: c * 128 + d_row + 1],
                                start=(c == 0), stop=(c == 1))
                nc.scalar.copy(out=ofm[:, b0:b0 + GB], in_=vps[:])

            nc.sync.dma_start(out=out_t[:, :], in_=ofm[:])
    nc.compile()
    return nc


def _lbt_perm():
    """Permutation: lbt row d  ->  group-local b index."""
    perm = np.zeros(GB, dtype=np.int64)
    for d in range(GB):
        u, dl = d // 16, d % 16
        pr, wn, h, cb = (dl // 8) % 2, (dl // 4) % 2, (dl // 2) % 2, dl % 2
        r = 8 * pr + 4 * h + 2 * wn + cb
        perm[d] = u * 16 + r
    return perm


def _host_prep(q, k, v, mask, W1, b1, W2, b2, Wf, bf):
    bf16 = ml_dtypes.bfloat16
    W1a, W1b = W1[0:D], W1[D:2 * D]
    W1c, W1d = W1[2 * D:3 * D], W1[3 * D:4 * D]
    ws = np.concatenate([W1b - W1c, W1d], axis=0).astype(bf16)       # [128, 80]
    wq = np.concatenate([W1a + W1c, b1[None, :]], axis=0).astype(bf16)  # [65, 80]
    w2 = np.zeros((H1, 64), dtype=np.float32)
    w2[:, 0:40] = W2
    w2 = w2.astype(bf16)
    wfd = np.zeros((104, 32), dtype=np.float32)
    wfd[0:40, 0] = Wf[:, 0]
    wfd[64:104, 1] = Wf[:, 0]
    wfd = wfd.astype(bf16)
    b2d = np.zeros((128, 1), dtype=np.float32)
    b2d[0:40, 0] = b2
    b2d[64:104, 0] = b2
    idm = np.eye(128, dtype=np.float32).astype(bf16)

    k_fm = np.zeros((B, D, TP), dtype=np.float32)
    k_fm[:, :, :T] = k.transpose(0, 2, 1)
    qk_fm = k_fm * q[:, :, None]
    kq = np.concatenate([k_fm, qk_fm], axis=1).astype(bf16)          # [B, 128, 256]
    vp = np.zeros((B, TP, D), dtype=np.float32)
    vp[:, :T, :] = v
    vp = vp.astype(bf16)
    mfp = np.zeros((B, TP), dtype=np.float32)
    mfp[:, :T] = (mask != 0).astype(np.float32)

    perm = _lbt_perm()
    in_maps = []
    for c in range(NCORES):
        s = slice(c * BC, (c + 1) * BC)
        q1 = np.concatenate(
            [q[s].T, np.ones((1, BC), np.float32)], axis=0).astype(bf16)
        mfc = mfp[s].reshape(NG, GB, TP)[:, perm, :].reshape(BC, TP)
        kqt = kq[s].reshape(32, 16, 128, TP).transpose(0, 2, 1, 3) \
            .reshape(32, 128, 16 * TP)
        vpt = vp[s].reshape(32, 16, 2, 128, D).transpose(0, 3, 1, 2, 4) \
            .reshape(32, 128, 16 * 128)
        in_maps.append({
            "kq": np.ascontiguousarray(kqt),
            "v": np.ascontiguousarray(vpt),
            "mf": np.ascontiguousarray(mfc),
            "q1": np.ascontiguousarray(q1),
            "ws": ws, "wq": wq, "w2": w2, "wf": wfd, "b2": b2d, "idm": idm,
        })
    return in_maps


def kernel(q, k, v, mask, W1, b1, W2, b2, Wf, bf, _trace=False):
    q = np.asarray(q, np.float32)
    k = np.asarray(k, np.float32)
    v = np.asarray(v, np.float32)
    mask = np.asarray(mask)
    in_maps = _host_prep(q, k, v, mask,
                         np.asarray(W1, np.float32), np.asarray(b1, np.float32),
                         np.asarray(W2, np.float32), np.asarray(b2, np.float32),
                         np.asarray(Wf, np.float32), np.asarray(bf, np.float32))
    if "nc" not in _cache:
        _cache["nc"] = _build_program()
    r = run_bass_kernel_spmd(_cache["nc"], in_maps,
                             core_ids=list(range(NCORES)), trace=_trace)
    out = np.concatenate([r.results[c]["ofm"].T for c in range(NCORES)], axis=0)
    if _trace:
        kernel.last_exec_ns = r.exec_time_ns
        kernel.last_results = r
    return out.astype(np.float32)
